# revision 12
# baseline (speedup 1.0000x reference)
# BiMamba Trainium2 kernel (Bass/Tile), self-contained.
#
# Problem: B=4, L=2048, D=256, 2 directions x 2 layers, d_inner=512,
# d_state=16, d_conv=4, dt_rank=16. Output (B, L, 2D) fp32.
#
# Sharding: 8 cores = (2 directions) x (4 batch samples); each core runs the
# full 2-layer stack for one (direction, sample) pair — zero collectives.
# Direction-1 cores get time-flipped input; their output is flipped back on
# the host.
#
# Per-core pipeline (all [partition, free] tiles, time on the free axis):
#   in_proj+conv: PE matmuls; the depthwise causal conv is folded into the
#       u-half in_proj as 4 time-shifted matmuls accumulating in PSUM
#       (weights pre-scaled by conv_w per tap on the host), evacuated through
#       ScalarE Silu (+conv bias) -> uc fp16.  z-half -> Silu -> siluz fp16.
#   x_proj: PE matmul -> (dt_raw fp16, B*(-2^14) fp16, C*2^-14 fp16).
#       The 2^14 keeps b/h inside fp16 normal range; the minus sign cancels
#       du' = -delta*uc below.
#   dt_proj: PE matmul; no HW softplus table, so
#       mdelta := -softplus(x) = ln(sigmoid(-x)) via Sigmoid+Ln.
#   volume loop (16 states n x 4 d-blocks of 128 channels):
#       a = Exp((n+1)*mdelta)          (ScalarE; (n+1) = exp(A_log[n]))
#       b = du' * B_bc[n]              (VectorE TT fp16 2x)
#       h = tensor_tensor_scan(a, b)   (VectorE; fp32 state, fp16 out)
#       g = h * C_bc[n]                (VectorE TT)
#       y += I.T @ g                   (PE identity-matmul accumulate)
#   skip/gate: y += diag(Dp) @ uc (PE); y_g = y * siluz (VectorE, PSUM src)
#   out_proj: PE -> [t, D] PSUM; LayerNorm via bn_stats/bn_aggr + Sqrt +
#       reciprocal + Identity-activation (per-partition scale/bias); layer
#       bridge via PE transpose back to [D, t].

import numpy as np

_CACHE = {}

B_, L_, D_ = 4, 2048, 256
DI, DS, DC, DTR = 512, 16, 4, 16
NL = 2
PAD = 4
B_SCALE = float(-(2.0 ** 14))
C_SCALE = float(2.0 ** -14)


def _install_ntff_hook():
    import sys, types
    if "antenv.axon_hooks" in sys.modules:
        return
    mod = types.ModuleType("antenv.axon_hooks")
    mod._hook = None
    mod.set_axon_ntff_profile_hook = lambda h: setattr(mod, "_hook", h)
    mod.get_axon_ntff_profile_hook = lambda: mod._hook
    sys.modules["antenv.axon_hooks"] = mod
    try:
        import antenv
        antenv.axon_hooks = mod
        from trn_agent_boot.trn_boot import _ntff_profile_via_ctypes
        mod.set_axon_ntff_profile_hook(
            _ntff_profile_via_ctypes("/opt/axon/libaxon_pjrt.so"))
    except Exception:
        pass


def _build(a_scales):
    import concourse.bass as bass
    import concourse.bacc as bacc
    import concourse.tile as tile
    import concourse.mybir as mybir
    from contextlib import ExitStack

    F32 = mybir.dt.float32
    F16 = mybir.dt.float16
    AF = mybir.ActivationFunctionType
    ALU = mybir.AluOpType
    L = L_

    nc = bacc.Bacc("TRN2", target_bir_lowering=False, debug=False)

    x_pad = nc.dram_tensor("x_pad", [D_, PAD + L], F16, kind="ExternalInput").ap()
    ins = {}
    for i in range(NL):
        ins[f"wu{i}"] = nc.dram_tensor(f"wu{i}", [128, 2, DC, DI], F16, kind="ExternalInput").ap()
        ins[f"wz{i}"] = nc.dram_tensor(f"wz{i}", [128, 2, DI], F16, kind="ExternalInput").ap()
        ins[f"cb{i}"] = nc.dram_tensor(f"cb{i}", [128, 4], F32, kind="ExternalInput").ap()
        ins[f"xw{i}"] = nc.dram_tensor(f"xw{i}", [128, 4, 96], F16, kind="ExternalInput").ap()
        ins[f"dtw{i}"] = nc.dram_tensor(f"dtw{i}", [DTR, DI], F16, kind="ExternalInput").ap()
        ins[f"ndtb{i}"] = nc.dram_tensor(f"ndtb{i}", [128, 4], F32, kind="ExternalInput").ap()
        ins[f"dpd{i}"] = nc.dram_tensor(f"dpd{i}", [128, 4, 128], F16, kind="ExternalInput").ap()
        ins[f"ow{i}"] = nc.dram_tensor(f"ow{i}", [128, 4, D_], F16, kind="ExternalInput").ap()
        ins[f"nw{i}"] = nc.dram_tensor(f"nw{i}", [1, D_], F32, kind="ExternalInput").ap()
        ins[f"nb{i}"] = nc.dram_tensor(f"nb{i}", [1, D_], F32, kind="ExternalInput").ap()
    ident_d = nc.dram_tensor("ident", [128, 128], F16, kind="ExternalInput").ap()
    out_d = nc.dram_tensor("out", [L, D_], F32, kind="ExternalOutput").ap()

    NT = L // 128
    NG = DI // 128

    with tile.TileContext(nc) as tc, ExitStack() as ctx:
        const_p = ctx.enter_context(tc.tile_pool(name="const", bufs=1))
        w_p = ctx.enter_context(tc.tile_pool(name="weights", bufs=1))
        maps_p = ctx.enter_context(tc.tile_pool(name="maps", bufs=1))
        vol_p = ctx.enter_context(tc.tile_pool(name="vol", bufs=2))
        bc_p = ctx.enter_context(tc.tile_pool(name="bc", bufs=3))
        small_p = ctx.enter_context(tc.tile_pool(name="small", bufs=2))
        q_p = ctx.enter_context(tc.tile_pool(name="qpool", bufs=2))
        dram_p = ctx.enter_context(tc.tile_pool(name="drams", bufs=1, space="DRAM"))
        ps_mm = ctx.enter_context(tc.tile_pool(name="psmm", bufs=2, space="PSUM"))
        ps_y = ctx.enter_context(tc.tile_pool(name="psy", bufs=1, space="PSUM"))

        ident = const_p.tile([128, 128], F16)
        nc.sync.dma_start(ident, ident_d)
        eps_t = const_p.tile([128, 1], F32)
        nc.vector.memset(eps_t, 1e-5)

        xt = [const_p.tile([128, PAD + L], F16, name=f"xt{j}", tag=f"xt{j}") for j in range(2)]
        xs = [const_p.tile([128, PAD + L], F16, name=f"xs{j}", tag=f"xs{j}") for j in range(2)]
        for j in range(2):
            nc.sync.dma_start(xt[j], x_pad[j * 128:(j + 1) * 128, :])
            # xs[:, c] = xt[:, c+1] so odd tap offsets become even
            nc.sync.dma_start(xs[j][:, 0:PAD + L - 1], x_pad[j * 128:(j + 1) * 128, 1:])
            nc.vector.memset(xs[j][:, PAD + L - 1:PAD + L], 0.0)

        for li in range(NL):
            wu = w_p.tile([128, 2, DC, DI], F16, tag="wu")
            for gg in range(4):
                nc.sync.dma_start(wu[:, :, :, gg * 128:(gg + 1) * 128],
                                  ins[f"wu{li}"][:, :, :, gg * 128:(gg + 1) * 128])
            wz = w_p.tile([128, 2, DI], F16, tag="wz")
            nc.sync.dma_start(wz, ins[f"wz{li}"])
            cb = w_p.tile([128, 4], F32, tag="cb")
            nc.sync.dma_start(cb, ins[f"cb{li}"])
            xw = w_p.tile([128, 4, 96], F16, tag="xw")
            nc.sync.dma_start(xw, ins[f"xw{li}"])
            dtw = w_p.tile([DTR, DI], F16, tag="dtw")
            nc.sync.dma_start(dtw, ins[f"dtw{li}"])
            ndtb = w_p.tile([128, 4], F32, tag="ndtb")
            nc.sync.dma_start(ndtb, ins[f"ndtb{li}"])
            dpd = w_p.tile([128, 4, 128], F16, tag="dpd")
            nc.sync.dma_start(dpd, ins[f"dpd{li}"])
            ow = w_p.tile([128, 4, D_], F16, tag="ow")
            nc.sync.dma_start(ow, ins[f"ow{li}"])
            nw_bc = w_p.tile([128, D_], F32, tag="nw")
            nc.sync.dma_start(nw_bc, bass.AP(
                tensor=ins[f"nw{li}"].tensor, offset=ins[f"nw{li}"].offset,
                ap=[[0, 128], [1, D_]]))
            nb_bc = w_p.tile([128, D_], F32, tag="nb")
            nc.sync.dma_start(nb_bc, bass.AP(
                tensor=ins[f"nb{li}"].tensor, offset=ins[f"nb{li}"].offset,
                ap=[[0, 128], [1, D_]]))

            # ---- P1: in_proj (+conv) -> uc ; z -> siluz
            uc = [maps_p.tile([128, L], F16, name=f"uc{g}", tag=f"uc{g}") for g in range(NG)]
            siluz = [maps_p.tile([128, L], F16, name=f"sz{g}", tag=f"sz{g}") for g in range(NG)]
            for g in range(NG):
                for nch in range(4):
                    t0 = nch * 512
                    pm = ps_mm.tile([128, 512], F32, tag="mm")
                    nmm = 0
                    for k in range(DC):
                        off = PAD - 3 + k  # 1..4
                        for kc in range(2):
                            if off % 2 == 1:
                                src = xs[kc][:, (off - 1) + t0:(off - 1) + t0 + 512]
                            else:
                                src = xt[kc][:, off + t0:off + t0 + 512]
                            nc.tensor.matmul(
                                pm, wu[:, kc, k, g * 128:(g + 1) * 128], src,
                                start=(nmm == 0), stop=(nmm == DC * 2 - 1))
                            nmm += 1
                    nc.scalar.activation(
                        uc[g][:, t0:t0 + 512], pm, AF.Silu,
                        bias=cb[:, g:g + 1], scale=1.0)
                    pz = ps_mm.tile([128, 512], F32, tag="mm")
                    for kc in range(2):
                        nc.tensor.matmul(
                            pz, wz[:, kc, g * 128:(g + 1) * 128],
                            xt[kc][:, PAD + t0:PAD + t0 + 512],
                            start=(kc == 0), stop=(kc == 1))
                    nc.scalar.activation(siluz[g][:, t0:t0 + 512], pz, AF.Silu)

            # ---- P2: x_proj -> dt_raw, B, C (B/C bounced to DRAM for bcast)
            # x_proj output padded to 96 rows: dt@0, B@32, C@64 (32-aligned
            # sections; B/C pre-scaled in the host weights). One ACT copy
            # evacuates PSUM; B/C rows bounce to DRAM for row-broadcasts.
            xdbl = maps_p.tile([96, L], F16, tag="xdbl")
            B_dr = dram_p.tile([DS, L], F16, tag="Bdr")
            C_dr = dram_p.tile([DS, L], F16, tag="Cdr")
            for ncy in range(4):
                t0 = ncy * 512
                px = ps_mm.tile([96, 512], F32, tag="mm")
                for kc in range(NG):
                    nc.tensor.matmul(
                        px, xw[:, kc, :], uc[kc][:, t0:t0 + 512],
                        start=(kc == 0), stop=(kc == NG - 1))
                nc.scalar.activation(xdbl[:, t0:t0 + 512], px, AF.Copy)
            nc.sync.dma_start(B_dr, xdbl[32:48, :])
            nc.sync.dma_start(C_dr, xdbl[64:80, :])
            dtraw = xdbl

            # ---- P3: dt_proj -> mdelta = ln(sigmoid(-(raw + dtb)))
            # All sigmoids batched (into the du tiles as fp16 scratch), then
            # all Lns — avoids ACT table-set thrash. du is rewritten in P4.
            mdelta = [maps_p.tile([128, L], F16, name=f"md{g}", tag=f"md{g}") for g in range(NG)]
            du = [maps_p.tile([128, L], F16, name=f"du{g}", tag=f"du{g}") for g in range(NG)]
            for g in range(NG):
                for ncy in range(4):
                    t0 = ncy * 512
                    pd = ps_mm.tile([128, 512], F32, tag="mm")
                    nc.tensor.matmul(pd, dtw[:, g * 128:(g + 1) * 128],
                                     dtraw[0:DTR, t0:t0 + 512], start=True, stop=True)
                    nc.scalar.activation(du[g][:, t0:t0 + 512], pd, AF.Sigmoid,
                                         bias=ndtb[:, g:g + 1], scale=-1.0)
            for g in range(NG):
                nc.scalar.activation(mdelta[g], du[g], AF.Ln)

            # ---- P4: du' = mdelta * uc (overwrites the sigmoid scratch)
            for g in range(NG):
                nc.vector.tensor_tensor(du[g], mdelta[g], uc[g], ALU.mult)

            # ---- P5: volume loop
            y_g = [maps_p.tile([128, L], F16, name=f"yg{g}", tag=f"yg{g}") for g in range(NG)]
            for g in range(NG):
                yp = ps_y.tile([128, L], F32, tag="y")
                for n in range(DS):
                    B_bc = bc_p.tile([128, L], F16, tag="Bbc")
                    nc.sync.dma_start(B_bc, bass.AP(
                        tensor=B_dr.tensor, offset=B_dr[n:n + 1, :].offset,
                        ap=[[0, 128], [1, L]]))
                    C_bc = bc_p.tile([128, L], F16, tag="Cbc")
                    nc.sync.dma_start(C_bc, bass.AP(
                        tensor=C_dr.tensor, offset=C_dr[n:n + 1, :].offset,
                        ap=[[0, 128], [1, L]]))
                    a_t = vol_p.tile([128, L], F16, tag="a")
                    nc.scalar.activation(a_t, mdelta[g], AF.Exp,
                                         scale=float(a_scales[n]))
                    b_t = vol_p.tile([128, L], F16, tag="b")
                    nc.vector.tensor_tensor(b_t, du[g], B_bc, ALU.mult)
                    h_t = vol_p.tile([128, L], F16, tag="h")
                    nc.vector.tensor_tensor_scan(h_t, a_t, b_t, 0.0,
                                                 ALU.mult, ALU.add)
                    g_t = vol_p.tile([128, L], F16, tag="g")
                    nc.vector.tensor_tensor(g_t, h_t, C_bc, ALU.mult)
                    for ncy in range(4):
                        nc.tensor.matmul(
                            yp[:, ncy * 512:(ncy + 1) * 512], ident,
                            g_t[:, ncy * 512:(ncy + 1) * 512],
                            start=(n == 0), stop=False)
                for ncy in range(4):
                    nc.tensor.matmul(
                        yp[:, ncy * 512:(ncy + 1) * 512], dpd[:, g, :],
                        uc[g][:, ncy * 512:(ncy + 1) * 512],
                        start=False, stop=(ncy == 3))
                ysb = vol_p.tile([128, L], F16, tag="ysb")
                nc.scalar.activation(ysb, yp, AF.Copy)
                nc.vector.tensor_tensor(y_g[g], ysb, siluz[g], ALU.mult)

            # ---- P6/P7: out_proj + LayerNorm (+ bridge / output)
            last = (li == NL - 1)
            if not last:
                xt = [const_p.tile([128, PAD + L], F16, name=f"xt{j}_l{li + 1}", tag=f"xt{j}_l{li + 1}")
                      for j in range(2)]
                xs = [const_p.tile([128, PAD + L], F16, name=f"xs{j}_l{li + 1}", tag=f"xs{j}_l{li + 1}")
                      for j in range(2)]
                for j in range(2):
                    nc.vector.memset(xt[j][:, 0:PAD], 0.0)
            for it in range(NT):
                t0 = it * 128
                po = ps_mm.tile([128, D_], F32, tag="mm")
                for kc in range(NG):
                    nc.tensor.matmul(po, y_g[kc][:, t0:t0 + 128],
                                     ow[:, kc, :],
                                     start=(kc == 0), stop=(kc == NG - 1))
                stats = small_p.tile([128, 6], F32, tag="st")
                nc.vector.bn_stats(stats, po)
                mv = small_p.tile([128, 2], F32, tag="mv")
                nc.vector.bn_aggr(mv, stats)
                sd = small_p.tile([128, 1], F32, tag="sd")
                nc.scalar.activation(sd, mv[:, 1:2], AF.Sqrt, bias=eps_t, scale=1.0)
                rstd = small_p.tile([128, 1], F32, tag="rs")
                nc.vector.reciprocal(rstd, sd)
                nrm = small_p.tile([128, D_], F32, tag="nrm")
                rstd_bc = bass.AP(tensor=rstd.tensor, offset=rstd.offset,
                                  ap=[list(rstd.ap[0]), [0, D_]])
                nc.vector.scalar_tensor_tensor(nrm, po, mv[:, 0:1], rstd_bc,
                                               ALU.subtract, ALU.mult)
                if last:
                    ow_t = small_p.tile([128, D_], F32, tag="own")
                    nc.vector.tensor_tensor(ow_t, nrm, nw_bc, ALU.mult)
                    nc.vector.tensor_tensor(ow_t, ow_t, nb_bc, ALU.add)
                    nc.sync.dma_start(out_d[t0:t0 + 128, :], ow_t)
                else:
                    h16 = small_p.tile([128, D_], F16, tag="h16")
                    nc.vector.tensor_tensor(h16, nrm, nw_bc, ALU.mult)
                    nc.vector.tensor_tensor(h16, h16, nb_bc, ALU.add)
                    for j in range(2):
                        pt = ps_mm.tile([128, 128], F16, tag="mm")
                        nc.tensor.transpose(pt, h16[:, j * 128:(j + 1) * 128], ident)
                        nc.vector.tensor_copy(
                            xt[j][:, PAD + t0:PAD + t0 + 128], pt)
            if not last:
                for j in range(2):
                    for c in range(4):
                        c0 = c * 512
                        ce = min(c0 + 512 + PAD, PAD + L) - 1
                        nc.sync.dma_start(xs[j][:, c0:ce], xt[j][:, c0 + 1:ce + 1])
                    nc.vector.memset(xs[j][:, PAD + L - 1:PAD + L], 0.0)

    nc.compile()
    return nc


def kernel(**inputs):
    _install_ntff_hook()
    from concourse.bass_utils import run_bass_kernel_spmd

    trace = bool(inputs.pop("_trace", False))

    x = np.asarray(inputs["x"], np.float32)
    in_w = np.asarray(inputs["in_proj_w"], np.float32)
    cw = np.asarray(inputs["conv_w"], np.float32)
    cbv = np.asarray(inputs["conv_b"], np.float32)
    xw = np.asarray(inputs["x_proj_w"], np.float32)
    dtw = np.asarray(inputs["dt_w"], np.float32)
    dtb = np.asarray(inputs["dt_b"], np.float32)
    Alog = np.asarray(inputs["A_log"], np.float32)
    Dp = np.asarray(inputs["Dp"], np.float32)
    owv = np.asarray(inputs["out_proj_w"], np.float32)
    nw = np.asarray(inputs["norm_w"], np.float32)
    nb = np.asarray(inputs["norm_b"], np.float32)

    a_scales = tuple(float(v) for v in np.exp(Alog[0, 0, 0]))

    key = (a_scales,)
    if key not in _CACHE:
        _CACHE[key] = _build(a_scales)
    nc = _CACHE[key]

    ident = np.eye(128, dtype=np.float16)
    in_maps = []
    for core in range(8):
        d, bi = core // 4, core % 4
        x_t = x[bi].T
        if d == 1:
            x_t = x_t[:, ::-1]
        xp = np.zeros((D_, PAD + L_), np.float16)
        xp[:, PAD:] = x_t.astype(np.float16)
        m = {"x_pad": xp, "ident": ident}
        for i in range(NL):
            wiu = in_w[d, i, :DI, :]                       # (DI, D)
            # wu[p, kc, k, e] = in_w_u[e, kc*128+p] * cw[e, k]
            wuk = np.stack([(wiu * cw[d, i, :, k:k + 1]).T for k in range(DC)],
                           axis=1)                         # (D, DC, DI)
            m[f"wu{i}"] = np.ascontiguousarray(
                wuk.reshape(2, 128, DC, DI).transpose(1, 0, 2, 3)
            ).astype(np.float16)                           # (128, 2, DC, DI)
            wzt = in_w[d, i, DI:, :].T                     # (D, DI)
            m[f"wz{i}"] = np.ascontiguousarray(
                wzt.reshape(2, 128, DI).transpose(1, 0, 2)).astype(np.float16)
            m[f"cb{i}"] = np.ascontiguousarray(
                cbv[d, i].reshape(4, 128).T).astype(np.float32)   # (128, 4)
            xws = np.zeros((96, DI), np.float32)
            xws[0:DTR] = xw[d, i][0:DTR]
            xws[32:48] = xw[d, i][DTR:2 * DTR] * B_SCALE
            xws[64:80] = xw[d, i][2 * DTR:] * C_SCALE
            xwt = xws.T                                    # (DI, 96)
            m[f"xw{i}"] = np.ascontiguousarray(
                xwt.reshape(4, 128, 96).transpose(1, 0, 2)).astype(np.float16)
            m[f"dtw{i}"] = np.ascontiguousarray(
                dtw[d, i].T).astype(np.float16)            # (DTR, DI)
            m[f"ndtb{i}"] = np.ascontiguousarray(
                (-dtb[d, i]).reshape(4, 128).T).astype(np.float32)
            dpdiag = np.zeros((128, 4, 128), np.float16)
            for g in range(4):
                np.fill_diagonal(dpdiag[:, g, :], Dp[d, i, g * 128:(g + 1) * 128])
            m[f"dpd{i}"] = dpdiag
            owt = owv[d, i].T                              # (DI, D)
            m[f"ow{i}"] = np.ascontiguousarray(
                owt.reshape(4, 128, D_).transpose(1, 0, 2)).astype(np.float16)
            m[f"nw{i}"] = nw[d, i][None, :].astype(np.float32)
            m[f"nb{i}"] = nb[d, i][None, :].astype(np.float32)
        in_maps.append(m)

    res = run_bass_kernel_spmd(nc, in_maps, core_ids=list(range(8)),
                               trace=trace, trace_cores=[0] if trace else None)
    kernel.last_result = res

    out = np.empty((B_, L_, 2 * D_), np.float32)
    for core in range(8):
        d, bi = core // 4, core % 4
        o = res.results[core]["out"]
        if d == 1:
            o = o[::-1, :]
        out[bi, :, d * D_:(d + 1) * D_] = o
    return out


# revision 13
# speedup vs baseline: 1.0209x; 1.0209x over previous
# BiMamba Trainium2 kernel (Bass/Tile), self-contained.
#
# Problem: B=4, L=2048, D=256, 2 directions x 2 layers, d_inner=512,
# d_state=16, d_conv=4, dt_rank=16. Output (B, L, 2D) fp32.
#
# Sharding: 8 cores = (2 directions) x (4 batch samples); each core runs the
# full 2-layer stack for one (direction, sample) pair — zero collectives.
# Direction-1 cores get time-flipped input; their output is flipped back on
# the host.
#
# Per-core pipeline (all [partition, free] tiles, time on the free axis):
#   in_proj+conv: PE matmuls; the depthwise causal conv is folded into the
#       u-half in_proj as 4 time-shifted matmuls accumulating in PSUM
#       (weights pre-scaled by conv_w per tap on the host), evacuated through
#       ScalarE Silu (+conv bias) -> uc fp16.  z-half -> Silu -> siluz fp16.
#   x_proj: PE matmul -> (dt_raw fp16, B*(-2^14) fp16, C*2^-14 fp16).
#       The 2^14 keeps b/h inside fp16 normal range; the minus sign cancels
#       du' = -delta*uc below.
#   dt_proj: PE matmul; no HW softplus table, so
#       mdelta := -softplus(x) = ln(sigmoid(-x)) via Sigmoid+Ln.
#   volume loop (16 states n x 4 d-blocks of 128 channels):
#       a = Exp((n+1)*mdelta)          (ScalarE; (n+1) = exp(A_log[n]))
#       b = du' * B_bc[n]              (VectorE TT fp16 2x)
#       h = tensor_tensor_scan(a, b)   (VectorE; fp32 state, fp16 out)
#       g = h * C_bc[n]                (VectorE TT)
#       y += I.T @ g                   (PE identity-matmul accumulate)
#   skip/gate: y += diag(Dp) @ uc (PE); y_g = y * siluz (VectorE, PSUM src)
#   out_proj: PE -> [t, D] PSUM; LayerNorm via bn_stats/bn_aggr + Sqrt +
#       reciprocal + Identity-activation (per-partition scale/bias); layer
#       bridge via PE transpose back to [D, t].

import numpy as np

_CACHE = {}

B_, L_, D_ = 4, 2048, 256
DI, DS, DC, DTR = 512, 16, 4, 16
NL = 2
PAD = 4
B_SCALE = float(-(2.0 ** 14))
C_SCALE = float(2.0 ** -14)


def _install_ntff_hook():
    import sys, types
    if "antenv.axon_hooks" in sys.modules:
        return
    mod = types.ModuleType("antenv.axon_hooks")
    mod._hook = None
    mod.set_axon_ntff_profile_hook = lambda h: setattr(mod, "_hook", h)
    mod.get_axon_ntff_profile_hook = lambda: mod._hook
    sys.modules["antenv.axon_hooks"] = mod
    try:
        import antenv
        antenv.axon_hooks = mod
        from trn_agent_boot.trn_boot import _ntff_profile_via_ctypes
        mod.set_axon_ntff_profile_hook(
            _ntff_profile_via_ctypes("/opt/axon/libaxon_pjrt.so"))
    except Exception:
        pass


def _build(a_scales):
    import concourse.bass as bass
    import concourse.bacc as bacc
    import concourse.tile as tile
    import concourse.mybir as mybir
    from contextlib import ExitStack

    F32 = mybir.dt.float32
    F16 = mybir.dt.float16
    AF = mybir.ActivationFunctionType
    ALU = mybir.AluOpType
    L = L_

    nc = bacc.Bacc("TRN2", target_bir_lowering=False, debug=False)

    x_pad = nc.dram_tensor("x_pad", [D_, PAD + L], F16, kind="ExternalInput").ap()
    ins = {}
    for i in range(NL):
        ins[f"wu{i}"] = nc.dram_tensor(f"wu{i}", [128, 2, DC, DI], F16, kind="ExternalInput").ap()
        ins[f"wz{i}"] = nc.dram_tensor(f"wz{i}", [128, 2, DI], F16, kind="ExternalInput").ap()
        ins[f"cb{i}"] = nc.dram_tensor(f"cb{i}", [128, 4], F32, kind="ExternalInput").ap()
        ins[f"xw{i}"] = nc.dram_tensor(f"xw{i}", [128, 4, 96], F16, kind="ExternalInput").ap()
        ins[f"dtw{i}"] = nc.dram_tensor(f"dtw{i}", [DTR, DI], F16, kind="ExternalInput").ap()
        ins[f"ndtb{i}"] = nc.dram_tensor(f"ndtb{i}", [128, 4], F32, kind="ExternalInput").ap()
        ins[f"dpd{i}"] = nc.dram_tensor(f"dpd{i}", [128, 4, 128], F16, kind="ExternalInput").ap()
        ins[f"ow{i}"] = nc.dram_tensor(f"ow{i}", [128, 4, D_], F16, kind="ExternalInput").ap()
        ins[f"nw{i}"] = nc.dram_tensor(f"nw{i}", [1, D_], F32, kind="ExternalInput").ap()
        ins[f"nb{i}"] = nc.dram_tensor(f"nb{i}", [1, D_], F32, kind="ExternalInput").ap()
    ident_d = nc.dram_tensor("ident", [128, 128], F16, kind="ExternalInput").ap()
    out_d = nc.dram_tensor("out", [L, D_], F32, kind="ExternalOutput").ap()

    NT = L // 128
    NG = DI // 128

    with tile.TileContext(nc) as tc, ExitStack() as ctx:
        const_p = ctx.enter_context(tc.tile_pool(name="const", bufs=1))
        w_p = ctx.enter_context(tc.tile_pool(name="weights", bufs=1))
        maps_p = ctx.enter_context(tc.tile_pool(name="maps", bufs=1))
        vol_p = ctx.enter_context(tc.tile_pool(name="vol", bufs=2))
        bc_p = ctx.enter_context(tc.tile_pool(name="bc", bufs=3))
        small_p = ctx.enter_context(tc.tile_pool(name="small", bufs=2))
        q_p = ctx.enter_context(tc.tile_pool(name="qpool", bufs=2))
        dram_p = ctx.enter_context(tc.tile_pool(name="drams", bufs=1, space="DRAM"))
        ps_mm = ctx.enter_context(tc.tile_pool(name="psmm", bufs=2, space="PSUM"))
        ps_y = ctx.enter_context(tc.tile_pool(name="psy", bufs=1, space="PSUM"))

        ident = const_p.tile([128, 128], F16)
        nc.sync.dma_start(ident, ident_d)
        eps_t = const_p.tile([128, 1], F32)
        nc.vector.memset(eps_t, 1e-5)

        xt = [const_p.tile([128, PAD + L], F16, name=f"xt{j}", tag=f"xt{j}") for j in range(2)]
        xs = [const_p.tile([128, PAD + L], F16, name=f"xs{j}", tag=f"xs{j}") for j in range(2)]
        for j in range(2):
            nc.sync.dma_start(xt[j], x_pad[j * 128:(j + 1) * 128, :])
            # xs[:, c] = xt[:, c+1] so odd tap offsets become even
            nc.sync.dma_start(xs[j][:, 0:PAD + L - 1], x_pad[j * 128:(j + 1) * 128, 1:])
            nc.vector.memset(xs[j][:, PAD + L - 1:PAD + L], 0.0)

        for li in range(NL):
            wu = w_p.tile([128, 2, DC, DI], F16, tag="wu")
            for gg in range(4):
                nc.sync.dma_start(wu[:, :, :, gg * 128:(gg + 1) * 128],
                                  ins[f"wu{li}"][:, :, :, gg * 128:(gg + 1) * 128])
            wz = w_p.tile([128, 2, DI], F16, tag="wz")
            nc.sync.dma_start(wz, ins[f"wz{li}"])
            cb = w_p.tile([128, 4], F32, tag="cb")
            nc.sync.dma_start(cb, ins[f"cb{li}"])
            xw = w_p.tile([128, 4, 96], F16, tag="xw")
            nc.sync.dma_start(xw, ins[f"xw{li}"])
            dtw = w_p.tile([DTR, DI], F16, tag="dtw")
            nc.sync.dma_start(dtw, ins[f"dtw{li}"])
            ndtb = w_p.tile([128, 4], F32, tag="ndtb")
            nc.sync.dma_start(ndtb, ins[f"ndtb{li}"])
            dpd = w_p.tile([128, 4, 128], F16, tag="dpd")
            nc.sync.dma_start(dpd, ins[f"dpd{li}"])
            ow = w_p.tile([128, 4, D_], F16, tag="ow")
            nc.sync.dma_start(ow, ins[f"ow{li}"])
            nw_bc = w_p.tile([128, D_], F32, tag="nw")
            nc.sync.dma_start(nw_bc, bass.AP(
                tensor=ins[f"nw{li}"].tensor, offset=ins[f"nw{li}"].offset,
                ap=[[0, 128], [1, D_]]))
            nb_bc = w_p.tile([128, D_], F32, tag="nb")
            nc.sync.dma_start(nb_bc, bass.AP(
                tensor=ins[f"nb{li}"].tensor, offset=ins[f"nb{li}"].offset,
                ap=[[0, 128], [1, D_]]))

            # ---- P1: in_proj (+conv) -> uc ; z -> siluz
            uc = [maps_p.tile([128, L], F16, name=f"uc{g}", tag=f"uc{g}") for g in range(NG)]
            siluz = [maps_p.tile([128, L], F16, name=f"sz{g}", tag=f"sz{g}") for g in range(NG)]
            for g in range(NG):
                for nch in range(4):
                    t0 = nch * 512
                    pm = ps_mm.tile([128, 512], F32, tag="mm")
                    nmm = 0
                    for k in range(DC):
                        off = PAD - 3 + k  # 1..4
                        for kc in range(2):
                            if off % 2 == 1:
                                src = xs[kc][:, (off - 1) + t0:(off - 1) + t0 + 512]
                            else:
                                src = xt[kc][:, off + t0:off + t0 + 512]
                            nc.tensor.matmul(
                                pm, wu[:, kc, k, g * 128:(g + 1) * 128], src,
                                start=(nmm == 0), stop=(nmm == DC * 2 - 1))
                            nmm += 1
                    nc.scalar.activation(
                        uc[g][:, t0:t0 + 512], pm, AF.Silu,
                        bias=cb[:, g:g + 1], scale=1.0)

            # ---- P2: x_proj -> dt_raw, B, C (B/C bounced to DRAM for bcast)
            # x_proj output padded to 96 rows: dt@0, B@32, C@64 (32-aligned
            # sections; B/C pre-scaled in the host weights). One ACT copy
            # evacuates PSUM; B/C rows bounce to DRAM for row-broadcasts.
            xdbl = maps_p.tile([96, L], F16, tag="xdbl")
            B_dr = dram_p.tile([DS, L], F16, tag="Bdr")
            C_dr = dram_p.tile([DS, L], F16, tag="Cdr")
            for ncy in range(4):
                t0 = ncy * 512
                px = ps_mm.tile([96, 512], F32, tag="mm")
                for kc in range(NG):
                    nc.tensor.matmul(
                        px, xw[:, kc, :], uc[kc][:, t0:t0 + 512],
                        start=(kc == 0), stop=(kc == NG - 1))
                nc.scalar.activation(xdbl[:, t0:t0 + 512], px, AF.Copy)
            nc.sync.dma_start(B_dr, xdbl[32:48, :])
            nc.sync.dma_start(C_dr, xdbl[64:80, :])
            dtraw = xdbl

            # ---- P3: dt_proj -> mdelta = ln(sigmoid(-(raw + dtb)))
            # All sigmoids batched (into the du tiles as fp16 scratch), then
            # all Lns — avoids ACT table-set thrash. du is rewritten in P4.
            mdelta = [maps_p.tile([128, L], F16, name=f"md{g}", tag=f"md{g}") for g in range(NG)]
            du = [maps_p.tile([128, L], F16, name=f"du{g}", tag=f"du{g}") for g in range(NG)]
            for g in range(NG):
                for ncy in range(4):
                    t0 = ncy * 512
                    pd = ps_mm.tile([128, 512], F32, tag="mm")
                    nc.tensor.matmul(pd, dtw[:, g * 128:(g + 1) * 128],
                                     dtraw[0:DTR, t0:t0 + 512], start=True, stop=True)
                    nc.scalar.activation(du[g][:, t0:t0 + 512], pd, AF.Sigmoid,
                                         bias=ndtb[:, g:g + 1], scale=-1.0)
            for g in range(NG):
                nc.scalar.activation(mdelta[g], du[g], AF.Ln)

            # ---- P4: du' = mdelta * uc (overwrites the sigmoid scratch)
            for g in range(NG):
                nc.vector.tensor_tensor(du[g], mdelta[g], uc[g], ALU.mult)

            # ---- z-half (deferred: only needed at the gate; runs under P5)
            for g in range(NG):
                for nch in range(4):
                    t0 = nch * 512
                    pz = ps_mm.tile([128, 512], F32, tag="mm")
                    for kc in range(2):
                        nc.tensor.matmul(
                            pz, wz[:, kc, g * 128:(g + 1) * 128],
                            xt[kc][:, PAD + t0:PAD + t0 + 512],
                            start=(kc == 0), stop=(kc == 1))
                    nc.scalar.activation(siluz[g][:, t0:t0 + 512], pz, AF.Silu)

            # ---- P5: volume loop
            y_g = [maps_p.tile([128, L], F16, name=f"yg{g}", tag=f"yg{g}") for g in range(NG)]
            for g in range(NG):
                yp = ps_y.tile([128, L], F32, tag="y")
                for n in range(DS):
                    B_bc = bc_p.tile([128, L], F16, tag="Bbc")
                    nc.sync.dma_start(B_bc, bass.AP(
                        tensor=B_dr.tensor, offset=B_dr[n:n + 1, :].offset,
                        ap=[[0, 128], [1, L]]))
                    C_bc = bc_p.tile([128, L], F16, tag="Cbc")
                    nc.sync.dma_start(C_bc, bass.AP(
                        tensor=C_dr.tensor, offset=C_dr[n:n + 1, :].offset,
                        ap=[[0, 128], [1, L]]))
                    a_t = vol_p.tile([128, L], F16, tag="a")
                    nc.scalar.activation(a_t, mdelta[g], AF.Exp,
                                         scale=float(a_scales[n]))
                    b_t = vol_p.tile([128, L], F16, tag="b")
                    nc.vector.tensor_tensor(b_t, du[g], B_bc, ALU.mult)
                    h_t = vol_p.tile([128, L], F16, tag="h")
                    nc.vector.tensor_tensor_scan(h_t, a_t, b_t, 0.0,
                                                 ALU.mult, ALU.add)
                    g_t = vol_p.tile([128, L], F16, tag="g")
                    nc.vector.tensor_tensor(g_t, h_t, C_bc, ALU.mult)
                    for ncy in range(4):
                        nc.tensor.matmul(
                            yp[:, ncy * 512:(ncy + 1) * 512], ident,
                            g_t[:, ncy * 512:(ncy + 1) * 512],
                            start=(n == 0), stop=False)
                for ncy in range(4):
                    nc.tensor.matmul(
                        yp[:, ncy * 512:(ncy + 1) * 512], dpd[:, g, :],
                        uc[g][:, ncy * 512:(ncy + 1) * 512],
                        start=False, stop=(ncy == 3))
                ysb = vol_p.tile([128, L], F16, tag="ysb")
                nc.scalar.activation(ysb, yp, AF.Copy)
                nc.vector.tensor_tensor(y_g[g], ysb, siluz[g], ALU.mult)

            # ---- P6/P7: out_proj + LayerNorm (+ bridge / output)
            last = (li == NL - 1)
            if not last:
                xt = [const_p.tile([128, PAD + L], F16, name=f"xt{j}_l{li + 1}", tag=f"xt{j}_l{li + 1}")
                      for j in range(2)]
                xs = [const_p.tile([128, PAD + L], F16, name=f"xs{j}_l{li + 1}", tag=f"xs{j}_l{li + 1}")
                      for j in range(2)]
                for j in range(2):
                    nc.vector.memset(xt[j][:, 0:PAD], 0.0)
            for it in range(NT):
                t0 = it * 128
                po = ps_mm.tile([128, D_], F32, tag="mm")
                for kc in range(NG):
                    nc.tensor.matmul(po, y_g[kc][:, t0:t0 + 128],
                                     ow[:, kc, :],
                                     start=(kc == 0), stop=(kc == NG - 1))
                stats = small_p.tile([128, 6], F32, tag="st")
                nc.vector.bn_stats(stats, po)
                mv = small_p.tile([128, 2], F32, tag="mv")
                nc.vector.bn_aggr(mv, stats)
                sd = small_p.tile([128, 1], F32, tag="sd")
                nc.scalar.activation(sd, mv[:, 1:2], AF.Sqrt, bias=eps_t, scale=1.0)
                rstd = small_p.tile([128, 1], F32, tag="rs")
                nc.vector.reciprocal(rstd, sd)
                nrm = small_p.tile([128, D_], F32, tag="nrm")
                rstd_bc = bass.AP(tensor=rstd.tensor, offset=rstd.offset,
                                  ap=[list(rstd.ap[0]), [0, D_]])
                nc.vector.scalar_tensor_tensor(nrm, po, mv[:, 0:1], rstd_bc,
                                               ALU.subtract, ALU.mult)
                if last:
                    ow_t = small_p.tile([128, D_], F32, tag="own")
                    nc.vector.tensor_tensor(ow_t, nrm, nw_bc, ALU.mult)
                    nc.vector.tensor_tensor(ow_t, ow_t, nb_bc, ALU.add)
                    nc.sync.dma_start(out_d[t0:t0 + 128, :], ow_t)
                else:
                    h16 = small_p.tile([128, D_], F16, tag="h16")
                    nc.vector.tensor_tensor(h16, nrm, nw_bc, ALU.mult)
                    nc.vector.tensor_tensor(h16, h16, nb_bc, ALU.add)
                    for j in range(2):
                        pt = ps_mm.tile([128, 128], F16, tag="mm")
                        nc.tensor.transpose(pt, h16[:, j * 128:(j + 1) * 128], ident)
                        nc.vector.tensor_copy(
                            xt[j][:, PAD + t0:PAD + t0 + 128], pt)
            if not last:
                for j in range(2):
                    for c in range(4):
                        c0 = c * 512
                        ce = min(c0 + 512 + PAD, PAD + L) - 1
                        nc.sync.dma_start(xs[j][:, c0:ce], xt[j][:, c0 + 1:ce + 1])
                    nc.vector.memset(xs[j][:, PAD + L - 1:PAD + L], 0.0)

    nc.compile()
    return nc


def kernel(**inputs):
    _install_ntff_hook()
    from concourse.bass_utils import run_bass_kernel_spmd

    trace = bool(inputs.pop("_trace", False))

    x = np.asarray(inputs["x"], np.float32)
    in_w = np.asarray(inputs["in_proj_w"], np.float32)
    cw = np.asarray(inputs["conv_w"], np.float32)
    cbv = np.asarray(inputs["conv_b"], np.float32)
    xw = np.asarray(inputs["x_proj_w"], np.float32)
    dtw = np.asarray(inputs["dt_w"], np.float32)
    dtb = np.asarray(inputs["dt_b"], np.float32)
    Alog = np.asarray(inputs["A_log"], np.float32)
    Dp = np.asarray(inputs["Dp"], np.float32)
    owv = np.asarray(inputs["out_proj_w"], np.float32)
    nw = np.asarray(inputs["norm_w"], np.float32)
    nb = np.asarray(inputs["norm_b"], np.float32)

    a_scales = tuple(float(v) for v in np.exp(Alog[0, 0, 0]))

    key = (a_scales,)
    if key not in _CACHE:
        _CACHE[key] = _build(a_scales)
    nc = _CACHE[key]

    ident = np.eye(128, dtype=np.float16)
    in_maps = []
    for core in range(8):
        d, bi = core // 4, core % 4
        x_t = x[bi].T
        if d == 1:
            x_t = x_t[:, ::-1]
        xp = np.zeros((D_, PAD + L_), np.float16)
        xp[:, PAD:] = x_t.astype(np.float16)
        m = {"x_pad": xp, "ident": ident}
        for i in range(NL):
            wiu = in_w[d, i, :DI, :]                       # (DI, D)
            # wu[p, kc, k, e] = in_w_u[e, kc*128+p] * cw[e, k]
            wuk = np.stack([(wiu * cw[d, i, :, k:k + 1]).T for k in range(DC)],
                           axis=1)                         # (D, DC, DI)
            m[f"wu{i}"] = np.ascontiguousarray(
                wuk.reshape(2, 128, DC, DI).transpose(1, 0, 2, 3)
            ).astype(np.float16)                           # (128, 2, DC, DI)
            wzt = in_w[d, i, DI:, :].T                     # (D, DI)
            m[f"wz{i}"] = np.ascontiguousarray(
                wzt.reshape(2, 128, DI).transpose(1, 0, 2)).astype(np.float16)
            m[f"cb{i}"] = np.ascontiguousarray(
                cbv[d, i].reshape(4, 128).T).astype(np.float32)   # (128, 4)
            xws = np.zeros((96, DI), np.float32)
            xws[0:DTR] = xw[d, i][0:DTR]
            xws[32:48] = xw[d, i][DTR:2 * DTR] * B_SCALE
            xws[64:80] = xw[d, i][2 * DTR:] * C_SCALE
            xwt = xws.T                                    # (DI, 96)
            m[f"xw{i}"] = np.ascontiguousarray(
                xwt.reshape(4, 128, 96).transpose(1, 0, 2)).astype(np.float16)
            m[f"dtw{i}"] = np.ascontiguousarray(
                dtw[d, i].T).astype(np.float16)            # (DTR, DI)
            m[f"ndtb{i}"] = np.ascontiguousarray(
                (-dtb[d, i]).reshape(4, 128).T).astype(np.float32)
            dpdiag = np.zeros((128, 4, 128), np.float16)
            for g in range(4):
                np.fill_diagonal(dpdiag[:, g, :], Dp[d, i, g * 128:(g + 1) * 128])
            m[f"dpd{i}"] = dpdiag
            owt = owv[d, i].T                              # (DI, D)
            m[f"ow{i}"] = np.ascontiguousarray(
                owt.reshape(4, 128, D_).transpose(1, 0, 2)).astype(np.float16)
            m[f"nw{i}"] = nw[d, i][None, :].astype(np.float32)
            m[f"nb{i}"] = nb[d, i][None, :].astype(np.float32)
        in_maps.append(m)

    res = run_bass_kernel_spmd(nc, in_maps, core_ids=list(range(8)),
                               trace=trace, trace_cores=[0] if trace else None)
    kernel.last_result = res

    out = np.empty((B_, L_, 2 * D_), np.float32)
    for core in range(8):
        d, bi = core // 4, core % 4
        o = res.results[core]["out"]
        if d == 1:
            o = o[::-1, :]
        out[bi, :, d * D_:(d + 1) * D_] = o
    return out


# revision 14
# speedup vs baseline: 1.0214x; 1.0004x over previous
# BiMamba Trainium2 kernel (Bass/Tile), self-contained.
#
# Problem: B=4, L=2048, D=256, 2 directions x 2 layers, d_inner=512,
# d_state=16, d_conv=4, dt_rank=16. Output (B, L, 2D) fp32.
#
# Sharding: 8 cores = (2 directions) x (4 batch samples); each core runs the
# full 2-layer stack for one (direction, sample) pair — zero collectives.
# Direction-1 cores get time-flipped input; their output is flipped back on
# the host.
#
# Per-core pipeline (all [partition, free] tiles, time on the free axis):
#   in_proj+conv: PE matmuls; the depthwise causal conv is folded into the
#       u-half in_proj as 4 time-shifted matmuls accumulating in PSUM
#       (weights pre-scaled by conv_w per tap on the host), evacuated through
#       ScalarE Silu (+conv bias) -> uc fp16.  z-half -> Silu -> siluz fp16.
#   x_proj: PE matmul -> (dt_raw fp16, B*(-2^14) fp16, C*2^-14 fp16).
#       The 2^14 keeps b/h inside fp16 normal range; the minus sign cancels
#       du' = -delta*uc below.
#   dt_proj: PE matmul; no HW softplus table, so
#       mdelta := -softplus(x) = ln(sigmoid(-x)) via Sigmoid+Ln.
#   volume loop (16 states n x 4 d-blocks of 128 channels):
#       a = Exp((n+1)*mdelta)          (ScalarE; (n+1) = exp(A_log[n]))
#       b = du' * B_bc[n]              (VectorE TT fp16 2x)
#       h = tensor_tensor_scan(a, b)   (VectorE; fp32 state, fp16 out)
#       g = h * C_bc[n]                (VectorE TT)
#       y += I.T @ g                   (PE identity-matmul accumulate)
#   skip/gate: y += diag(Dp) @ uc (PE); y_g = y * siluz (VectorE, PSUM src)
#   out_proj: PE -> [t, D] PSUM; LayerNorm via bn_stats/bn_aggr + Sqrt +
#       reciprocal + Identity-activation (per-partition scale/bias); layer
#       bridge via PE transpose back to [D, t].

import numpy as np

_CACHE = {}

B_, L_, D_ = 4, 2048, 256
DI, DS, DC, DTR = 512, 16, 4, 16
NL = 2
PAD = 4
B_SCALE = float(-(2.0 ** 14))
C_SCALE = float(2.0 ** -14)


def _install_ntff_hook():
    import sys, types
    if "antenv.axon_hooks" in sys.modules:
        return
    mod = types.ModuleType("antenv.axon_hooks")
    mod._hook = None
    mod.set_axon_ntff_profile_hook = lambda h: setattr(mod, "_hook", h)
    mod.get_axon_ntff_profile_hook = lambda: mod._hook
    sys.modules["antenv.axon_hooks"] = mod
    try:
        import antenv
        antenv.axon_hooks = mod
        from trn_agent_boot.trn_boot import _ntff_profile_via_ctypes
        mod.set_axon_ntff_profile_hook(
            _ntff_profile_via_ctypes("/opt/axon/libaxon_pjrt.so"))
    except Exception:
        pass


def _build(a_scales):
    import concourse.bass as bass
    import concourse.bacc as bacc
    import concourse.tile as tile
    import concourse.mybir as mybir
    from contextlib import ExitStack

    F32 = mybir.dt.float32
    F16 = mybir.dt.float16
    AF = mybir.ActivationFunctionType
    ALU = mybir.AluOpType
    L = L_

    nc = bacc.Bacc("TRN2", target_bir_lowering=False, debug=False)

    x_pad = nc.dram_tensor("x_pad", [D_, PAD + L], F16, kind="ExternalInput").ap()
    ins = {}
    for i in range(NL):
        ins[f"wu{i}"] = nc.dram_tensor(f"wu{i}", [128, 2, DC, DI], F16, kind="ExternalInput").ap()
        ins[f"wz{i}"] = nc.dram_tensor(f"wz{i}", [128, 2, DI], F16, kind="ExternalInput").ap()
        ins[f"cb{i}"] = nc.dram_tensor(f"cb{i}", [128, 4], F32, kind="ExternalInput").ap()
        ins[f"xw{i}"] = nc.dram_tensor(f"xw{i}", [128, 4, 96], F16, kind="ExternalInput").ap()
        ins[f"dtw{i}"] = nc.dram_tensor(f"dtw{i}", [DTR, DI], F16, kind="ExternalInput").ap()
        ins[f"ndtb{i}"] = nc.dram_tensor(f"ndtb{i}", [128, 4], F32, kind="ExternalInput").ap()
        ins[f"dpd{i}"] = nc.dram_tensor(f"dpd{i}", [128, 4, 128], F16, kind="ExternalInput").ap()
        ins[f"ow{i}"] = nc.dram_tensor(f"ow{i}", [128, 4, D_], F16, kind="ExternalInput").ap()
        ins[f"nw{i}"] = nc.dram_tensor(f"nw{i}", [1, D_], F32, kind="ExternalInput").ap()
        ins[f"nb{i}"] = nc.dram_tensor(f"nb{i}", [1, D_], F32, kind="ExternalInput").ap()
    ident_d = nc.dram_tensor("ident", [128, 128], F16, kind="ExternalInput").ap()
    out_d = nc.dram_tensor("out", [L, D_], F32, kind="ExternalOutput").ap()

    NT = L // 128
    NG = DI // 128

    with tile.TileContext(nc) as tc, ExitStack() as ctx:
        const_p = ctx.enter_context(tc.tile_pool(name="const", bufs=1))
        w_p = ctx.enter_context(tc.tile_pool(name="weights", bufs=1))
        maps_p = ctx.enter_context(tc.tile_pool(name="maps", bufs=1))
        vol_p = ctx.enter_context(tc.tile_pool(name="vol", bufs=2))
        bc_p = ctx.enter_context(tc.tile_pool(name="bc", bufs=3))
        small_p = ctx.enter_context(tc.tile_pool(name="small", bufs=2))
        q_p = ctx.enter_context(tc.tile_pool(name="qpool", bufs=2))
        dram_p = ctx.enter_context(tc.tile_pool(name="drams", bufs=1, space="DRAM"))
        ps_mm = ctx.enter_context(tc.tile_pool(name="psmm", bufs=2, space="PSUM"))
        ps_y = ctx.enter_context(tc.tile_pool(name="psy", bufs=1, space="PSUM"))

        ident = const_p.tile([128, 128], F16)
        nc.sync.dma_start(ident, ident_d)
        eps_t = const_p.tile([128, 1], F32)
        nc.vector.memset(eps_t, 1e-5)

        xt = [const_p.tile([128, PAD + L], F16, name=f"xt{j}", tag=f"xt{j}") for j in range(2)]
        xs = [const_p.tile([128, PAD + L], F16, name=f"xs{j}", tag=f"xs{j}") for j in range(2)]
        for j in range(2):
            nc.sync.dma_start(xt[j], x_pad[j * 128:(j + 1) * 128, :])
            # xs[:, c] = xt[:, c+1] so odd tap offsets become even
            nc.sync.dma_start(xs[j][:, 0:PAD + L - 1], x_pad[j * 128:(j + 1) * 128, 1:])
            nc.vector.memset(xs[j][:, PAD + L - 1:PAD + L], 0.0)

        for li in range(NL):
            wu = w_p.tile([128, 2, DC, DI], F16, tag="wu")
            for gg in range(4):
                nc.sync.dma_start(wu[:, :, :, gg * 128:(gg + 1) * 128],
                                  ins[f"wu{li}"][:, :, :, gg * 128:(gg + 1) * 128])
            wz = w_p.tile([128, 2, DI], F16, tag="wz")
            nc.sync.dma_start(wz, ins[f"wz{li}"])
            cb = w_p.tile([128, 4], F32, tag="cb")
            nc.sync.dma_start(cb, ins[f"cb{li}"])
            xw = w_p.tile([128, 4, 96], F16, tag="xw")
            nc.sync.dma_start(xw, ins[f"xw{li}"])
            dtw = w_p.tile([DTR, DI], F16, tag="dtw")
            nc.sync.dma_start(dtw, ins[f"dtw{li}"])
            ndtb = w_p.tile([128, 4], F32, tag="ndtb")
            nc.sync.dma_start(ndtb, ins[f"ndtb{li}"])
            dpd = w_p.tile([128, 4, 128], F16, tag="dpd")
            nc.sync.dma_start(dpd, ins[f"dpd{li}"])
            ow = w_p.tile([128, 4, D_], F16, tag="ow")
            nc.sync.dma_start(ow, ins[f"ow{li}"])
            nw_bc = w_p.tile([128, D_], F32, tag="nw")
            nc.sync.dma_start(nw_bc, bass.AP(
                tensor=ins[f"nw{li}"].tensor, offset=ins[f"nw{li}"].offset,
                ap=[[0, 128], [1, D_]]))
            nb_bc = w_p.tile([128, D_], F32, tag="nb")
            nc.sync.dma_start(nb_bc, bass.AP(
                tensor=ins[f"nb{li}"].tensor, offset=ins[f"nb{li}"].offset,
                ap=[[0, 128], [1, D_]]))

            # ---- P1: in_proj (+conv) -> uc ; z -> siluz
            uc = [maps_p.tile([128, L], F16, name=f"uc{g}", tag=f"uc{g}") for g in range(NG)]
            siluz = [maps_p.tile([128, L], F16, name=f"sz{g}", tag=f"sz{g}") for g in range(NG)]
            for nch in range(4):
                for g in range(NG):
                    t0 = nch * 512
                    pm = ps_mm.tile([128, 512], F32, tag="mm")
                    nmm = 0
                    for k in range(DC):
                        off = PAD - 3 + k  # 1..4
                        for kc in range(2):
                            if off % 2 == 1:
                                src = xs[kc][:, (off - 1) + t0:(off - 1) + t0 + 512]
                            else:
                                src = xt[kc][:, off + t0:off + t0 + 512]
                            nc.tensor.matmul(
                                pm, wu[:, kc, k, g * 128:(g + 1) * 128], src,
                                start=(nmm == 0), stop=(nmm == DC * 2 - 1))
                            nmm += 1
                    nc.scalar.activation(
                        uc[g][:, t0:t0 + 512], pm, AF.Silu,
                        bias=cb[:, g:g + 1], scale=1.0)

            # ---- P2: x_proj -> dt_raw, B, C (B/C bounced to DRAM for bcast)
            # x_proj output padded to 96 rows: dt@0, B@32, C@64 (32-aligned
            # sections; B/C pre-scaled in the host weights). One ACT copy
            # evacuates PSUM; B/C rows bounce to DRAM for row-broadcasts.
            xdbl = maps_p.tile([96, L], F16, tag="xdbl")
            B_dr = dram_p.tile([DS, L], F16, tag="Bdr")
            C_dr = dram_p.tile([DS, L], F16, tag="Cdr")
            for ncy in range(4):
                t0 = ncy * 512
                px = ps_mm.tile([96, 512], F32, tag="mm")
                for kc in range(NG):
                    nc.tensor.matmul(
                        px, xw[:, kc, :], uc[kc][:, t0:t0 + 512],
                        start=(kc == 0), stop=(kc == NG - 1))
                nc.scalar.activation(xdbl[:, t0:t0 + 512], px, AF.Copy)
            nc.sync.dma_start(B_dr, xdbl[32:48, :])
            nc.sync.dma_start(C_dr, xdbl[64:80, :])
            dtraw = xdbl

            # ---- P3: dt_proj -> mdelta = ln(sigmoid(-(raw + dtb)))
            # All sigmoids batched (into the du tiles as fp16 scratch), then
            # all Lns — avoids ACT table-set thrash. du is rewritten in P4.
            mdelta = [maps_p.tile([128, L], F16, name=f"md{g}", tag=f"md{g}") for g in range(NG)]
            du = [maps_p.tile([128, L], F16, name=f"du{g}", tag=f"du{g}") for g in range(NG)]
            for g in range(NG):
                for ncy in range(4):
                    t0 = ncy * 512
                    pd = ps_mm.tile([128, 512], F32, tag="mm")
                    nc.tensor.matmul(pd, dtw[:, g * 128:(g + 1) * 128],
                                     dtraw[0:DTR, t0:t0 + 512], start=True, stop=True)
                    nc.scalar.activation(du[g][:, t0:t0 + 512], pd, AF.Sigmoid,
                                         bias=ndtb[:, g:g + 1], scale=-1.0)
            for g in range(NG):
                for ncy in range(4):
                    t0 = ncy * 512
                    nc.scalar.activation(mdelta[g][:, t0:t0 + 512],
                                         du[g][:, t0:t0 + 512], AF.Ln)

            # ---- P4: du' = mdelta * uc (overwrites the sigmoid scratch)
            for g in range(NG):
                for ncy in range(4):
                    t0 = ncy * 512
                    nc.vector.tensor_tensor(du[g][:, t0:t0 + 512],
                                            mdelta[g][:, t0:t0 + 512],
                                            uc[g][:, t0:t0 + 512], ALU.mult)

            # ---- z-half (deferred: only needed at the gate; runs under P5)
            for g in range(NG):
                for nch in range(4):
                    t0 = nch * 512
                    pz = ps_mm.tile([128, 512], F32, tag="mm")
                    for kc in range(2):
                        nc.tensor.matmul(
                            pz, wz[:, kc, g * 128:(g + 1) * 128],
                            xt[kc][:, PAD + t0:PAD + t0 + 512],
                            start=(kc == 0), stop=(kc == 1))
                    nc.scalar.activation(siluz[g][:, t0:t0 + 512], pz, AF.Silu)

            # ---- P5: volume loop
            y_g = [maps_p.tile([128, L], F16, name=f"yg{g}", tag=f"yg{g}") for g in range(NG)]
            for g in range(NG):
                yp = ps_y.tile([128, L], F32, tag="y")
                for n in range(DS):
                    B_bc = bc_p.tile([128, L], F16, tag="Bbc")
                    nc.sync.dma_start(B_bc, bass.AP(
                        tensor=B_dr.tensor, offset=B_dr[n:n + 1, :].offset,
                        ap=[[0, 128], [1, L]]))
                    C_bc = bc_p.tile([128, L], F16, tag="Cbc")
                    nc.sync.dma_start(C_bc, bass.AP(
                        tensor=C_dr.tensor, offset=C_dr[n:n + 1, :].offset,
                        ap=[[0, 128], [1, L]]))
                    a_t = vol_p.tile([128, L], F16, tag="a")
                    nc.scalar.activation(a_t, mdelta[g], AF.Exp,
                                         scale=float(a_scales[n]))
                    b_t = vol_p.tile([128, L], F16, tag="b")
                    nc.vector.tensor_tensor(b_t, du[g], B_bc, ALU.mult)
                    h_t = vol_p.tile([128, L], F16, tag="h")
                    nc.vector.tensor_tensor_scan(h_t, a_t, b_t, 0.0,
                                                 ALU.mult, ALU.add)
                    g_t = vol_p.tile([128, L], F16, tag="g")
                    nc.vector.tensor_tensor(g_t, h_t, C_bc, ALU.mult)
                    for ncy in range(4):
                        nc.tensor.matmul(
                            yp[:, ncy * 512:(ncy + 1) * 512], ident,
                            g_t[:, ncy * 512:(ncy + 1) * 512],
                            start=(n == 0), stop=False)
                for ncy in range(4):
                    nc.tensor.matmul(
                        yp[:, ncy * 512:(ncy + 1) * 512], dpd[:, g, :],
                        uc[g][:, ncy * 512:(ncy + 1) * 512],
                        start=False, stop=(ncy == 3))
                ysb = vol_p.tile([128, L], F16, tag="ysb")
                nc.scalar.activation(ysb, yp, AF.Copy)
                nc.vector.tensor_tensor(y_g[g], ysb, siluz[g], ALU.mult)

            # ---- P6/P7: out_proj + LayerNorm (+ bridge / output)
            last = (li == NL - 1)
            if not last:
                xt = [const_p.tile([128, PAD + L], F16, name=f"xt{j}_l{li + 1}", tag=f"xt{j}_l{li + 1}")
                      for j in range(2)]
                xs = [const_p.tile([128, PAD + L], F16, name=f"xs{j}_l{li + 1}", tag=f"xs{j}_l{li + 1}")
                      for j in range(2)]
                for j in range(2):
                    nc.vector.memset(xt[j][:, 0:PAD], 0.0)
            for it in range(NT):
                t0 = it * 128
                po = ps_mm.tile([128, D_], F32, tag="mm")
                for kc in range(NG):
                    nc.tensor.matmul(po, y_g[kc][:, t0:t0 + 128],
                                     ow[:, kc, :],
                                     start=(kc == 0), stop=(kc == NG - 1))
                stats = small_p.tile([128, 6], F32, tag="st")
                nc.vector.bn_stats(stats, po)
                mv = small_p.tile([128, 2], F32, tag="mv")
                nc.vector.bn_aggr(mv, stats)
                sd = small_p.tile([128, 1], F32, tag="sd")
                nc.scalar.activation(sd, mv[:, 1:2], AF.Sqrt, bias=eps_t, scale=1.0)
                rstd = small_p.tile([128, 1], F32, tag="rs")
                nc.vector.reciprocal(rstd, sd)
                nrm = small_p.tile([128, D_], F32, tag="nrm")
                rstd_bc = bass.AP(tensor=rstd.tensor, offset=rstd.offset,
                                  ap=[list(rstd.ap[0]), [0, D_]])
                nc.vector.scalar_tensor_tensor(nrm, po, mv[:, 0:1], rstd_bc,
                                               ALU.subtract, ALU.mult)
                if last:
                    ow_t = small_p.tile([128, D_], F32, tag="own")
                    nc.vector.tensor_tensor(ow_t, nrm, nw_bc, ALU.mult)
                    nc.vector.tensor_tensor(ow_t, ow_t, nb_bc, ALU.add)
                    nc.sync.dma_start(out_d[t0:t0 + 128, :], ow_t)
                else:
                    h16 = small_p.tile([128, D_], F16, tag="h16")
                    nc.vector.tensor_tensor(h16, nrm, nw_bc, ALU.mult)
                    nc.vector.tensor_tensor(h16, h16, nb_bc, ALU.add)
                    for j in range(2):
                        pt = ps_mm.tile([128, 128], F16, tag="mm")
                        nc.tensor.transpose(pt, h16[:, j * 128:(j + 1) * 128], ident)
                        nc.vector.tensor_copy(
                            xt[j][:, PAD + t0:PAD + t0 + 128], pt)
            if not last:
                for j in range(2):
                    for c in range(4):
                        c0 = c * 512
                        ce = min(c0 + 512 + PAD, PAD + L) - 1
                        nc.sync.dma_start(xs[j][:, c0:ce], xt[j][:, c0 + 1:ce + 1])
                    nc.vector.memset(xs[j][:, PAD + L - 1:PAD + L], 0.0)

    nc.compile()
    return nc


def kernel(**inputs):
    _install_ntff_hook()
    from concourse.bass_utils import run_bass_kernel_spmd

    trace = bool(inputs.pop("_trace", False))

    x = np.asarray(inputs["x"], np.float32)
    in_w = np.asarray(inputs["in_proj_w"], np.float32)
    cw = np.asarray(inputs["conv_w"], np.float32)
    cbv = np.asarray(inputs["conv_b"], np.float32)
    xw = np.asarray(inputs["x_proj_w"], np.float32)
    dtw = np.asarray(inputs["dt_w"], np.float32)
    dtb = np.asarray(inputs["dt_b"], np.float32)
    Alog = np.asarray(inputs["A_log"], np.float32)
    Dp = np.asarray(inputs["Dp"], np.float32)
    owv = np.asarray(inputs["out_proj_w"], np.float32)
    nw = np.asarray(inputs["norm_w"], np.float32)
    nb = np.asarray(inputs["norm_b"], np.float32)

    a_scales = tuple(float(v) for v in np.exp(Alog[0, 0, 0]))

    key = (a_scales,)
    if key not in _CACHE:
        _CACHE[key] = _build(a_scales)
    nc = _CACHE[key]

    ident = np.eye(128, dtype=np.float16)
    in_maps = []
    for core in range(8):
        d, bi = core // 4, core % 4
        x_t = x[bi].T
        if d == 1:
            x_t = x_t[:, ::-1]
        xp = np.zeros((D_, PAD + L_), np.float16)
        xp[:, PAD:] = x_t.astype(np.float16)
        m = {"x_pad": xp, "ident": ident}
        for i in range(NL):
            wiu = in_w[d, i, :DI, :]                       # (DI, D)
            # wu[p, kc, k, e] = in_w_u[e, kc*128+p] * cw[e, k]
            wuk = np.stack([(wiu * cw[d, i, :, k:k + 1]).T for k in range(DC)],
                           axis=1)                         # (D, DC, DI)
            m[f"wu{i}"] = np.ascontiguousarray(
                wuk.reshape(2, 128, DC, DI).transpose(1, 0, 2, 3)
            ).astype(np.float16)                           # (128, 2, DC, DI)
            wzt = in_w[d, i, DI:, :].T                     # (D, DI)
            m[f"wz{i}"] = np.ascontiguousarray(
                wzt.reshape(2, 128, DI).transpose(1, 0, 2)).astype(np.float16)
            m[f"cb{i}"] = np.ascontiguousarray(
                cbv[d, i].reshape(4, 128).T).astype(np.float32)   # (128, 4)
            xws = np.zeros((96, DI), np.float32)
            xws[0:DTR] = xw[d, i][0:DTR]
            xws[32:48] = xw[d, i][DTR:2 * DTR] * B_SCALE
            xws[64:80] = xw[d, i][2 * DTR:] * C_SCALE
            xwt = xws.T                                    # (DI, 96)
            m[f"xw{i}"] = np.ascontiguousarray(
                xwt.reshape(4, 128, 96).transpose(1, 0, 2)).astype(np.float16)
            m[f"dtw{i}"] = np.ascontiguousarray(
                dtw[d, i].T).astype(np.float16)            # (DTR, DI)
            m[f"ndtb{i}"] = np.ascontiguousarray(
                (-dtb[d, i]).reshape(4, 128).T).astype(np.float32)
            dpdiag = np.zeros((128, 4, 128), np.float16)
            for g in range(4):
                np.fill_diagonal(dpdiag[:, g, :], Dp[d, i, g * 128:(g + 1) * 128])
            m[f"dpd{i}"] = dpdiag
            owt = owv[d, i].T                              # (DI, D)
            m[f"ow{i}"] = np.ascontiguousarray(
                owt.reshape(4, 128, D_).transpose(1, 0, 2)).astype(np.float16)
            m[f"nw{i}"] = nw[d, i][None, :].astype(np.float32)
            m[f"nb{i}"] = nb[d, i][None, :].astype(np.float32)
        in_maps.append(m)

    res = run_bass_kernel_spmd(nc, in_maps, core_ids=list(range(8)),
                               trace=trace, trace_cores=[0] if trace else None)
    kernel.last_result = res

    out = np.empty((B_, L_, 2 * D_), np.float32)
    for core in range(8):
        d, bi = core // 4, core % 4
        o = res.results[core]["out"]
        if d == 1:
            o = o[::-1, :]
        out[bi, :, d * D_:(d + 1) * D_] = o
    return out
